# revision 12
# baseline (speedup 1.0000x reference)
"""LIF spike kernel for Trainium2 (Bass/Tile), data-parallel over batch on 8 cores.

Host layout per core: x_core [C=128, B_loc=4, T*HW=8192] f32; per (b, t) the
compute tile is [128, 1024] (HW columns).

State is u_t (pre-reset membrane); the hard reset folds into the next step:
  u_t     = select(u_{t-1} <= 1, u_{t-1}, 0) * 0.5 + x_t   (custom DVE op, 1 op/step)
  spike_t = sign(u_t - 1) saturated to uint8               (ACT engine, exact {0,1})

sign->u8 saturates negatives to 0 (HW-verified), so spike == (u > 1) exactly,
including u == 1 (sign(0) = 0). All arithmetic is bit-exact fp32 vs the
reference (mult by 0.5 exact, one rounding add, select exact).

DMA plan: inputs stream as 16 x 1 MB chunks (one per (b, t-pair), 8 KB per
partition) alternated across the two HWDGE rings (sync/scalar) at high
priority; the [128,1024] compute chain rides the stream with ~2 us lag.
Outputs accumulate into one [128, 8192] u8 tile per b and ship as 4 x 1 MB
transfers split across both rings; ring FIFOs drain them after the inputs.
"""

import numpy as np

import concourse.bacc as bacc
import concourse.mybir as mybir
import concourse.dve_ops as dve_ops
from concourse.dve_ops import DveOp
from concourse.dve_spec import Spec, Src0, Src1, C0, C1, Zero, select, lower, _has_src1
from concourse.dve_uop import DveOpSpec
from concourse.dve_table_gen import dve_ver_for
from concourse.tile import TileContext
from concourse.bass_utils import run_bass_kernel_spmd

B, T, C, H, W = 32, 8, 128, 32, 32
HW = H * W
N_CORES = 8
B_LOC = B // N_CORES
THW = T * HW  # 8192
TAU = 0.5
THRESH = 1.0

_nc_cache = None


def _register_lif_step():
    name = "LIF_STEP_ANT"
    for op in dve_ops.OPS:
        if op.name == name:
            return op

    def _ref(in0, in1, s0, s1, imm2):
        m = np.where(
            np.asarray(in0, np.float32) <= np.float32(s1), in0, np.float32(0.0)
        ).astype(np.float32)
        return (m * np.float32(s0) + np.asarray(in1, np.float32)).astype(np.float32)

    body = select(Src0 <= C1, Src0, Zero) * C0 + Src1
    spec = Spec(body=body, reference=_ref)
    row = dve_ops._CUSTOM_DVE_ROW_BASE + len(dve_ops.OPS)
    ver = dve_ver_for("TRN2")
    tmp = DveOpSpec(name=name, opcode=row, uops=lower(spec, ver=ver), rd1_en=_has_src1(spec))
    op = DveOp(name, spec, subdim=False, uops_sha={ver: tmp.sha(ver)})
    dve_ops.OPS.append(op)
    dve_ops._SUB_OPCODE_FOR_NAME[name] = row
    dve_ops.CUSTOM_DVE_SPECS[name] = spec
    return op


def build_nc():
    lif_op = _register_lif_step()
    nc = bacc.Bacc("TRN2", target_bir_lowering=False)
    f32 = mybir.dt.float32
    u8 = mybir.dt.uint8
    act = mybir.ActivationFunctionType

    x = nc.dram_tensor("x", [C, B_LOC, THW], f32, kind="ExternalInput")
    out = nc.dram_tensor("out", [C, B_LOC, THW], u8, kind="ExternalOutput")

    # input chunks in consumption order, alternating rings; b0 split finer
    # so the compute chain starts as early as possible. (t-ranges per b)
    CHUNKS = [(0, 0, 2), (0, 2, 4), (0, 4, 8)] + [
        (b, lo, lo + 4) for b in range(1, B_LOC) for lo in (0, 4)
    ]

    with TileContext(nc) as tc:
        with (
            tc.tile_pool(name="xp", bufs=B_LOC) as xp,
            tc.tile_pool(name="up", bufs=3) as up,
            tc.tile_pool(name="sp", bufs=2) as sp,
            tc.tile_pool(name="cp", bufs=1) as cp,
        ):
            negone = cp.tile([C, 1], f32, tag="negone")
            nc.gpsimd.memset(negone[:], -1.0)

            xb = [
                xp.tile([C, THW], f32, tag="x", name=f"xb{b}") for b in range(B_LOC)
            ]
            with tc.high_priority():
                for k, (b, tlo, thi) in enumerate(CHUNKS):
                    eng = nc.sync if k % 2 == 0 else nc.scalar
                    eng.dma_start(
                        out=xb[b][:, tlo * HW : thi * HW],
                        in_=x[:, b, tlo * HW : thi * HW],
                    )

            for b in range(B_LOC):
                sb = sp.tile([C, THW], u8, tag="s")
                u_prev = None
                for t in range(T):
                    xs = xb[b][:, t * HW : (t + 1) * HW]
                    if t == 0:
                        u = xs
                    else:
                        u = up.tile([C, HW], f32, tag="u")
                        nc.vector._custom_dve(
                            lif_op, out=u[:], in0=u_prev[:], in1=xs,
                            s0=TAU, s1=THRESH,
                        )
                    nc.scalar.activation(
                        sb[:, t * HW : (t + 1) * HW],
                        u if t == 0 else u[:],
                        act.Sign, bias=negone[:],
                    )
                    u_prev = u
                # 1 MB output per b. Early b's dispatch from the scalar
                # engine mid-stream; late b's from the idle sync engine so
                # the final output is never queued behind remaining
                # ACTIVATEs. Ring FIFOs drain outputs after inputs.
                eng = nc.scalar if b < 2 else nc.sync
                eng.dma_start(out=out[:, b, :], in_=sb[:])
    nc.compile()
    return nc


def make_in_maps(x: np.ndarray) -> list[dict]:
    # x [B,T,C,H,W] -> per core [C, B_loc, T*HW]
    xs = np.ascontiguousarray(x).reshape(B, T, C, HW)
    return [
        {
            "x": np.ascontiguousarray(
                xs[i * B_LOC : (i + 1) * B_LOC].transpose(2, 0, 1, 3)
            ).reshape(C, B_LOC, THW)
        }
        for i in range(N_CORES)
    ]


def kernel(x: np.ndarray) -> np.ndarray:
    global _nc_cache
    if _nc_cache is None:
        _nc_cache = build_nc()
    res = run_bass_kernel_spmd(_nc_cache, make_in_maps(x), list(range(N_CORES)))
    # out[c, b_loc, t*HW+hw] -> [b, t, c, hw]
    parts = [
        res.results[i]["out"].reshape(C, B_LOC, T, HW).transpose(1, 2, 0, 3)
        for i in range(N_CORES)
    ]
    full = np.concatenate(parts, axis=0)
    return full.reshape(B, T, C, H, W).astype(np.float32)


# revision 13
# speedup vs baseline: 1.1594x; 1.1594x over previous
"""LIF spike kernel for Trainium2 (Bass/Tile), data-parallel over batch on 8 cores.

Host layout per core: x_core [C=128, B_loc=4, T*HW=8192] f32; per (b, t) the
compute tile is [128, 1024] (HW columns).

State is u_t (pre-reset membrane); the hard reset folds into the next step:
  u_t     = select(u_{t-1} <= 1, u_{t-1}, 0) * 0.5 + x_t   (custom DVE op, 1 op/step)
  spike_t = sign(u_t - 1) saturated to uint8               (ACT engine, exact {0,1})

sign->u8 saturates negatives to 0 (HW-verified), so spike == (u > 1) exactly,
including u == 1 (sign(0) = 0). All arithmetic is bit-exact fp32 vs the
reference (mult by 0.5 exact, one rounding add, select exact).

DMA plan: inputs stream as 16 x 1 MB chunks (one per (b, t-pair), 8 KB per
partition) alternated across the two HWDGE rings (sync/scalar) at high
priority; the [128,1024] compute chain rides the stream with ~2 us lag.
Outputs accumulate into one [128, 8192] u8 tile per b and ship as 4 x 1 MB
transfers split across both rings; ring FIFOs drain them after the inputs.
"""

import numpy as np

import concourse.bacc as bacc
import concourse.mybir as mybir
import concourse.dve_ops as dve_ops
from concourse.dve_ops import DveOp
from concourse.dve_spec import Spec, Src0, Src1, C0, C1, Zero, select, lower, _has_src1
from concourse.dve_uop import DveOpSpec
from concourse.dve_table_gen import dve_ver_for
from concourse.tile import TileContext
from concourse.bass_utils import run_bass_kernel_spmd

B, T, C, H, W = 32, 8, 128, 32, 32
HW = H * W
N_CORES = 8
B_LOC = B // N_CORES
THW = T * HW  # 8192
TAU = 0.5
THRESH = 1.0

_nc_cache = None


def _register_lif_step():
    name = "LIF_STEP_ANT"
    for op in dve_ops.OPS:
        if op.name == name:
            return op

    def _ref(in0, in1, s0, s1, imm2):
        m = np.where(
            np.asarray(in0, np.float32) <= np.float32(s1), in0, np.float32(0.0)
        ).astype(np.float32)
        return (m * np.float32(s0) + np.asarray(in1, np.float32)).astype(np.float32)

    body = select(Src0 <= C1, Src0, Zero) * C0 + Src1
    spec = Spec(body=body, reference=_ref)
    row = dve_ops._CUSTOM_DVE_ROW_BASE + len(dve_ops.OPS)
    ver = dve_ver_for("TRN2")
    tmp = DveOpSpec(name=name, opcode=row, uops=lower(spec, ver=ver), rd1_en=_has_src1(spec))
    op = DveOp(name, spec, subdim=False, uops_sha={ver: tmp.sha(ver)})
    dve_ops.OPS.append(op)
    dve_ops._SUB_OPCODE_FOR_NAME[name] = row
    dve_ops.CUSTOM_DVE_SPECS[name] = spec
    return op


def build_nc():
    lif_op = _register_lif_step()
    nc = bacc.Bacc("TRN2", target_bir_lowering=False)
    f32 = mybir.dt.float32
    u8 = mybir.dt.uint8
    act = mybir.ActivationFunctionType

    x = nc.dram_tensor("x", [C, B_LOC, THW], f32, kind="ExternalInput")
    out = nc.dram_tensor("out", [C, B_LOC, THW], u8, kind="ExternalOutput")

    # input chunks in consumption order, alternating rings; b0 split finer
    # so the compute chain starts as early as possible. (t-ranges per b)
    CHUNKS = [(0, 0, 2), (0, 2, 4), (0, 4, 8)] + [
        (b, lo, lo + 4) for b in range(1, B_LOC) for lo in (0, 4)
    ]

    with TileContext(nc) as tc:
        with (
            tc.tile_pool(name="xp", bufs=B_LOC) as xp,
            tc.tile_pool(name="up", bufs=5) as up,
            tc.tile_pool(name="sp", bufs=3) as sp,
            tc.tile_pool(name="cp", bufs=1) as cp,
        ):
            negone = cp.tile([C, 1], f32, tag="negone")
            nc.gpsimd.memset(negone[:], -1.0)

            xb = [
                xp.tile([C, THW], f32, tag="x", name=f"xb{b}") for b in range(B_LOC)
            ]
            with tc.high_priority():
                for k, (b, tlo, thi) in enumerate(CHUNKS):
                    eng = nc.sync if k % 2 == 0 else nc.scalar
                    eng.dma_start(
                        out=xb[b][:, tlo * HW : thi * HW],
                        in_=x[:, b, tlo * HW : thi * HW],
                    )

            for b in range(B_LOC):
                sb = sp.tile([C, THW], u8, tag="s")
                u_prev = None
                for t in range(T):
                    xs = xb[b][:, t * HW : (t + 1) * HW]
                    if t == 0:
                        u = xs
                    else:
                        u = up.tile([C, HW], f32, tag="u")
                        nc.vector._custom_dve(
                            lif_op, out=u[:], in0=u_prev[:], in1=xs,
                            s0=TAU, s1=THRESH,
                        )
                    nc.scalar.activation(
                        sb[:, t * HW : (t + 1) * HW],
                        u if t == 0 else u[:],
                        act.Sign, bias=negone[:],
                    )
                    u_prev = u
                # 1 MB output per b. Early b's dispatch from the scalar
                # engine mid-stream; late b's from the idle sync engine so
                # the final output is never queued behind remaining
                # ACTIVATEs. Ring FIFOs drain outputs after inputs.
                eng = nc.scalar if b < 2 else nc.sync
                eng.dma_start(out=out[:, b, :], in_=sb[:])
    nc.compile()
    return nc


def make_in_maps(x: np.ndarray) -> list[dict]:
    # x [B,T,C,H,W] -> per core [C, B_loc, T*HW]
    xs = np.ascontiguousarray(x).reshape(B, T, C, HW)
    return [
        {
            "x": np.ascontiguousarray(
                xs[i * B_LOC : (i + 1) * B_LOC].transpose(2, 0, 1, 3)
            ).reshape(C, B_LOC, THW)
        }
        for i in range(N_CORES)
    ]


def kernel(x: np.ndarray) -> np.ndarray:
    global _nc_cache
    if _nc_cache is None:
        _nc_cache = build_nc()
    res = run_bass_kernel_spmd(_nc_cache, make_in_maps(x), list(range(N_CORES)))
    # out[c, b_loc, t*HW+hw] -> [b, t, c, hw]
    parts = [
        res.results[i]["out"].reshape(C, B_LOC, T, HW).transpose(1, 2, 0, 3)
        for i in range(N_CORES)
    ]
    full = np.concatenate(parts, axis=0)
    return full.reshape(B, T, C, H, W).astype(np.float32)
